# revision 6
# baseline (speedup 1.0000x reference)
"""v8 = v6 + 3-ring x loads (sync 1.28 / scalar 0.64 / gpsimd 0.64 MB)
and 2/3/3 block stores, so ring totals stay 2.56 MB each while BOTH
the load phase and the store phase use all three DMA queues.
v6 = v4 + block-granular stores (8 x 0.64 MB, same ring balance).
v4 = v2 + balanced DMA rings (2.56 MB each on sync/scalar/gpsimd per
iteration) + block-granular first x load for an earlier compute start.

Math (separable linear):
    y[b, j] = w_x * x[b, j] + (nongene[b] . W_ng + bias) + (emb[j] . W_e)

Sharding: gene-parallel across 8 cores; each core handles a 2500-gene
column slice for the full 1024-row batch.

v2 vs baseline: the one-shot latency is dominated by the serialized DMA
stream (27 small DMAs on one HWDGE ring).  v2 cuts this to 9 DMAs spread
over all three DMA queues (qSPDynamicHW / qActDynamicHW / SWDGE):
  - 1 packed const load on the gpsimd (SWDGE) queue,
  - 4 x quarter-loads [128, 5000] fp8, alternating sync/act queues,
  - 4 y pair-stores [128, 5000] bf16 split sync/act/gpsimd queues,
    issued only at queue positions that never stall a compute engine.
x and y use a col-block = row-block packed DRAM layout ([128, 8*2500])
so row blocks are column ranges of one 128-partition tensor (pure host
layout transform).
"""

import numpy as np
import ml_dtypes
from contextlib import ExitStack

import concourse.bass as bass
import concourse.bacc as bacc
import concourse.tile as tile
from concourse import mybir
from concourse.bass_utils import run_bass_kernel_spmd

F32 = mybir.dt.float32
BF16 = mybir.dt.bfloat16
FP8 = mybir.dt.float8e3

NP_BF16 = ml_dtypes.bfloat16
NP_FP8 = ml_dtypes.float8_e3m4

B = 1024
G = 20000
DNG = 64
E = 32
FC_IN = DNG + 1 + E       # 97
NCORES = 8
GC = G // NCORES          # 2500 gene columns per core
PB = 128                  # SBUF partitions
RB = B // PB              # 8 row blocks per core
NQ = 4                    # x load quarters / y store pairs
QW = RB // NQ * GC        # 5000 cols per quarter/pair
SPLIT = 1536              # ACT handles [0:SPLIT), Pool [SPLIT:GC) per block

BANK = 512                # f32 columns per PSUM bank
NBANK = (GC + BANK - 1) // BANK

# const pack layout: [64, CW] bf16
#   rows 0:33, cols 0:GC        = [embT ; ones]
#   rows 0:33, cols GC:GC+PB    = [W_e | b] broadcast
#   rows 0:64, cols CX:CX+B     = xng.T
#   rows 0:64, col  CX+B        = W_ng
#   rows 0:64, col  CX+B+1      = w_x
#   rows 0:64, col  CX+B+2      = 1.0
CX = GC + PB              # 2628
CW = CX + B + 3           # 3655


def build_kernel(nc: bass.Bass, repeat: int = 1, serial: bool = False):
    xgd = nc.dram_tensor("xg", [PB, RB * GC], FP8, kind="ExternalInput").ap()
    cpkd = nc.dram_tensor("cpk", [DNG, CW], BF16, kind="ExternalInput").ap()
    ysd = nc.dram_tensor("ys", [PB, RB * GC], BF16, kind="ExternalOutput").ap()

    with tile.TileContext(nc) as tc, ExitStack() as ctx:
        const = ctx.enter_context(tc.tile_pool(name="const", bufs=1))
        psum = ctx.enter_context(tc.tile_pool(name="psum", bufs=1, space="PSUM"))
        xpool = ctx.enter_context(tc.tile_pool(name="xpool", bufs=NQ))
        ypool = ctx.enter_context(tc.tile_pool(name="ypool", bufs=NQ))

        # ---- dummy activation: hoists LoadActFuncSet to t~0 ----
        zin = const.tile([1, 2], F32)
        nc.gpsimd.memset(zin, 0.0)
        zout = const.tile([1, 2], F32)
        nc.scalar.activation(
            out=zout, in_=zin, func=mybir.ActivationFunctionType.Identity
        )

        # ---- const load on the SWDGE queue; x quarters on the two ----
        # ---- HWDGE queues, two each, so all three queues pull at t=0 ----
        cpk = const.tile([DNG, CW], BF16)
        nc.gpsimd.dma_start(out=cpk, in_=cpkd)
        embT = cpk[0:E + 1, 0:GC]               # [33, 2500] = [embT ; ones]
        web = cpk[0:E + 1, GC:GC + PB]          # [33, 128]  = [W_e | b] bcast
        xngT = cpk[:, CX:CX + B]                # [64, 1024] = xng.T
        wngcol = cpk[:, CX + B:CX + B + 1]      # [64, 1]    = W_ng

        gate = const.tile([1, RB], BF16)        # serial-mode barrier tile

        def issue_x_loads(first: bool):
            x_ts = []
            for q in range(NQ):
                x_t = xpool.tile([PB, QW], FP8, tag="x")
                x_ts.append(x_t)
            if serial and not first:
                # gate: strided read touching every stored region, then a
                # tiny DVE splash into each x tile corner.  Forces repeat
                # r's loads to wait for r-1's stores (one-shot latency
                # approximation for the repeat-slope measurement).
                nc.sync.dma_start(
                    out=gate, in_=ysd[0:1, GC - 2:RB * GC:GC]
                )
                for q in range(NQ):
                    nc.vector.tensor_copy(x_ts[q][0:1, 0:RB], gate)
            # loads use ALL THREE rings so the load phase takes 1/3 the
            # ring-time: q0 (split at block granularity for an early
            # compute start) + q3 on sync, q1 on scalar, q2 on gpsimd.
            nc.sync.dma_start(out=x_ts[0][:, 0:GC], in_=xgd[:, 0:GC])
            nc.sync.dma_start(out=x_ts[0][:, GC:QW], in_=xgd[:, GC:QW])
            nc.scalar.dma_start(out=x_ts[1], in_=xgd[:, QW:2 * QW])
            nc.gpsimd.dma_start(out=x_ts[2], in_=xgd[:, 2 * QW:3 * QW])
            nc.sync.dma_start(out=x_ts[3], in_=xgd[:, 3 * QW:4 * QW])
            return x_ts

        x_ts = issue_x_loads(first=True)

        # w_x broadcast across partitions: ones[1,128]^T @ wx[1,1]
        wxp = psum.tile([PB, 1], F32, tag="wx")
        nc.tensor.matmul(
            wxp,
            cpk[0:1, CX + B + 2:CX + B + 3].to_broadcast([1, PB]),
            cpk[0:1, CX + B + 1:CX + B + 2],
            start=True,
            stop=True,
        )
        wxc = const.tile([PB, 1], F32)
        nc.vector.tensor_copy(wxc, wxp)

        # ---- ng term on PE: ngp[p, a] = nongene[a*128+p] . W_ng ----
        ngp = psum.tile([PB, RB], F32, tag="ng")
        for a in range(RB):
            nc.tensor.matmul(
                ngp[:, a:a + 1],
                xngT[:, a * PB:(a + 1) * PB],
                wngcol,
                start=True,
                stop=True,
            )
        ngb = const.tile([PB, RB], F32)
        nc.vector.tensor_copy(ngb, ngp)

        # ---- gene term (+ fc bias): matmul + bf16 copy per bank ----
        grow = const.tile([PB, GC], BF16)
        for q in range(NBANK):
            c0 = q * BANK
            cw = min(BANK, GC - c0)
            gps = psum.tile([PB, BANK], F32, tag=f"g{q}")
            nc.tensor.matmul(
                gps[:, 0:cw],
                web,
                embT[:, c0:c0 + cw],
                start=True,
                stop=True,
            )
            nc.vector.tensor_copy(grow[:, c0:c0 + cw], gps[:, 0:cw])

        # ---- main stream: 8 row blocks in 4 quarter tiles ----
        # Block a lives in x_ts[a//2][:, (a%2)*GC :], y pair tiles hold
        # blocks (2k, 2k+1).  ACT does cols [0:SPLIT), Pool the rest,
        # DVE adds grow.  Stores: pair 0,2 -> sync queue (idle engine),
        # pair 1 -> act queue after its compute, pair 3 -> gpsimd queue
        # after its compute.
        lo = slice(0, SPLIT)
        hi = slice(SPLIT, GC)
        for r in range(repeat):
            if r > 0:
                x_ts = issue_x_loads(first=False)
            y_ts = []
            for k in range(NQ):
                y_t = ypool.tile([PB, QW], BF16, tag="y")
                y_ts.append(y_t)
            # phase 1: per-block scale-add on ACT (lo) and Pool (hi)
            for a in range(RB):
                x_blk = x_ts[a // 2][:, (a % 2) * GC:(a % 2 + 1) * GC]
                y_blk = y_ts[a // 2][:, (a % 2) * GC:(a % 2 + 1) * GC]
                nc.scalar.activation(
                    out=y_blk[:, lo],
                    in_=x_blk[:, lo],
                    func=mybir.ActivationFunctionType.Identity,
                    bias=ngb[:, a:a + 1],
                    scale=wxc,
                )
                nc.gpsimd.tensor_scalar(
                    out=y_blk[:, hi],
                    in0=x_blk[:, hi],
                    scalar1=wxc,
                    scalar2=ngb[:, a:a + 1],
                    op0=mybir.AluOpType.mult,
                    op1=mybir.AluOpType.add,
                )
            # phase 2: grow adds on DVE + block-granular stores.
            # Ring balance per iteration stays 2.56 MB each: blocks 0-1 on
            # sync, 2-3 on scalar, 4-7 on gpsimd; finer stores start the
            # write stream ~2us earlier and smooth the HBM demand.
            for a in range(RB):
                y_t = y_ts[a // 2]
                e = a % 2
                for sl in (lo, hi):
                    dst = slice(e * GC + sl.start, e * GC + sl.stop)
                    nc.vector.tensor_add(y_t[:, dst], y_t[:, dst], grow[:, sl])
                eng = nc.sync if a < 2 else (nc.scalar if a < 5 else nc.gpsimd)
                eng.dma_start(
                    out=ysd[:, a * GC:(a + 1) * GC],
                    in_=y_t[:, e * GC:(e + 1) * GC],
                )


def make_nc(repeat: int = 1, serial: bool = False) -> bacc.Bacc:
    nc = bacc.Bacc("TRN2", debug=False, num_devices=NCORES)
    build_kernel(nc, repeat=repeat, serial=serial)
    nc.compile()
    return nc


def prep_inputs(inputs) -> list:
    """Shard + downcast + repack the full inputs into per-core in_maps."""
    x = np.asarray(inputs["x"], dtype=np.float32)
    emb = np.asarray(inputs["emb"], dtype=np.float32)
    W = np.asarray(inputs["W"], dtype=np.float32).reshape(FC_IN)
    b = float(np.asarray(inputs["b"], dtype=np.float32).reshape(()))

    base = np.zeros((DNG, CW), dtype=np.float32)
    base[:, CX:CX + B] = x[:, G:].T
    base[:, CX + B] = W[0:DNG]
    base[:, CX + B + 1] = W[DNG]
    base[:, CX + B + 2] = 1.0
    base[0:E, GC:GC + PB] = W[DNG + 1:FC_IN, None]
    base[E, GC:GC + PB] = b

    in_maps = []
    for c in range(NCORES):
        sl = slice(c * GC, (c + 1) * GC)
        cpk = base.copy()
        cpk[0:E, 0:GC] = emb[sl].T
        cpk[E, 0:GC] = 1.0
        xg = (
            np.ascontiguousarray(x[:, sl])
            .reshape(RB, PB, GC)
            .transpose(1, 0, 2)
            .reshape(PB, RB * GC)
        )
        in_maps.append({
            "xg": np.ascontiguousarray(xg).astype(NP_FP8),
            "cpk": cpk.astype(NP_BF16),
        })
    return in_maps


def unshard(res_core: np.ndarray) -> np.ndarray:
    """[128, 8*2500] packed -> [1024, 2500] row-major (pure layout)."""
    return (
        np.asarray(res_core)
        .reshape(PB, RB, GC)
        .transpose(1, 0, 2)
        .reshape(B, GC)
        .astype(np.float32)
    )


def kernel(**inputs) -> np.ndarray:
    nc = make_nc()
    in_maps = prep_inputs(inputs)
    res = run_bass_kernel_spmd(nc, in_maps, core_ids=list(range(NCORES)))
    return np.concatenate(
        [unshard(r["ys"]) for r in res.results], axis=1
    )


# revision 7
# speedup vs baseline: 1.0517x; 1.0517x over previous
"""v12 = v8 with blocks 6+7 stored as one pair DMA on gpsimd (3 SWDGE
emissions per iteration instead of 4; same bytes per ring).
v8 = v6 + 3-ring x loads (sync 1.28 / scalar 0.64 / gpsimd 0.64 MB)
and 2/3/3 block stores, so ring totals stay 2.56 MB each while BOTH
the load phase and the store phase use all three DMA queues.
v6 = v4 + block-granular stores (8 x 0.64 MB, same ring balance).
v4 = v2 + balanced DMA rings (2.56 MB each on sync/scalar/gpsimd per
iteration) + block-granular first x load for an earlier compute start.

Math (separable linear):
    y[b, j] = w_x * x[b, j] + (nongene[b] . W_ng + bias) + (emb[j] . W_e)

Sharding: gene-parallel across 8 cores; each core handles a 2500-gene
column slice for the full 1024-row batch.

v2 vs baseline: the one-shot latency is dominated by the serialized DMA
stream (27 small DMAs on one HWDGE ring).  v2 cuts this to 9 DMAs spread
over all three DMA queues (qSPDynamicHW / qActDynamicHW / SWDGE):
  - 1 packed const load on the gpsimd (SWDGE) queue,
  - 4 x quarter-loads [128, 5000] fp8, alternating sync/act queues,
  - 4 y pair-stores [128, 5000] bf16 split sync/act/gpsimd queues,
    issued only at queue positions that never stall a compute engine.
x and y use a col-block = row-block packed DRAM layout ([128, 8*2500])
so row blocks are column ranges of one 128-partition tensor (pure host
layout transform).
"""

import numpy as np
import ml_dtypes
from contextlib import ExitStack

import concourse.bass as bass
import concourse.bacc as bacc
import concourse.tile as tile
from concourse import mybir
from concourse.bass_utils import run_bass_kernel_spmd

F32 = mybir.dt.float32
BF16 = mybir.dt.bfloat16
FP8 = mybir.dt.float8e3

NP_BF16 = ml_dtypes.bfloat16
NP_FP8 = ml_dtypes.float8_e3m4

B = 1024
G = 20000
DNG = 64
E = 32
FC_IN = DNG + 1 + E       # 97
NCORES = 8
GC = G // NCORES          # 2500 gene columns per core
PB = 128                  # SBUF partitions
RB = B // PB              # 8 row blocks per core
NQ = 4                    # x load quarters / y store pairs
QW = RB // NQ * GC        # 5000 cols per quarter/pair
SPLIT = 1536              # ACT handles [0:SPLIT), Pool [SPLIT:GC) per block

BANK = 512                # f32 columns per PSUM bank
NBANK = (GC + BANK - 1) // BANK

# const pack layout: [64, CW] bf16
#   rows 0:33, cols 0:GC        = [embT ; ones]
#   rows 0:33, cols GC:GC+PB    = [W_e | b] broadcast
#   rows 0:64, cols CX:CX+B     = xng.T
#   rows 0:64, col  CX+B        = W_ng
#   rows 0:64, col  CX+B+1      = w_x
#   rows 0:64, col  CX+B+2      = 1.0
CX = GC + PB              # 2628
CW = CX + B + 3           # 3655


def build_kernel(nc: bass.Bass, repeat: int = 1, serial: bool = False):
    xgd = nc.dram_tensor("xg", [PB, RB * GC], FP8, kind="ExternalInput").ap()
    cpkd = nc.dram_tensor("cpk", [DNG, CW], BF16, kind="ExternalInput").ap()
    ysd = nc.dram_tensor("ys", [PB, RB * GC], BF16, kind="ExternalOutput").ap()

    with tile.TileContext(nc) as tc, ExitStack() as ctx:
        const = ctx.enter_context(tc.tile_pool(name="const", bufs=1))
        psum = ctx.enter_context(tc.tile_pool(name="psum", bufs=1, space="PSUM"))
        xpool = ctx.enter_context(tc.tile_pool(name="xpool", bufs=NQ))
        ypool = ctx.enter_context(tc.tile_pool(name="ypool", bufs=NQ))

        # ---- dummy activation: hoists LoadActFuncSet to t~0 ----
        zin = const.tile([1, 2], F32)
        nc.gpsimd.memset(zin, 0.0)
        zout = const.tile([1, 2], F32)
        nc.scalar.activation(
            out=zout, in_=zin, func=mybir.ActivationFunctionType.Identity
        )

        # ---- const load on the SWDGE queue; x quarters on the two ----
        # ---- HWDGE queues, two each, so all three queues pull at t=0 ----
        cpk = const.tile([DNG, CW], BF16)
        nc.gpsimd.dma_start(out=cpk, in_=cpkd)
        embT = cpk[0:E + 1, 0:GC]               # [33, 2500] = [embT ; ones]
        web = cpk[0:E + 1, GC:GC + PB]          # [33, 128]  = [W_e | b] bcast
        xngT = cpk[:, CX:CX + B]                # [64, 1024] = xng.T
        wngcol = cpk[:, CX + B:CX + B + 1]      # [64, 1]    = W_ng

        gate = const.tile([1, RB], BF16)        # serial-mode barrier tile

        def issue_x_loads(first: bool):
            x_ts = []
            for q in range(NQ):
                x_t = xpool.tile([PB, QW], FP8, tag="x")
                x_ts.append(x_t)
            if serial and not first:
                # gate: strided read touching every stored region, then a
                # tiny DVE splash into each x tile corner.  Forces repeat
                # r's loads to wait for r-1's stores (one-shot latency
                # approximation for the repeat-slope measurement).
                nc.sync.dma_start(
                    out=gate, in_=ysd[0:1, GC - 2:RB * GC:GC]
                )
                for q in range(NQ):
                    nc.vector.tensor_copy(x_ts[q][0:1, 0:RB], gate)
            # loads use ALL THREE rings so the load phase takes 1/3 the
            # ring-time: q0 (split at block granularity for an early
            # compute start) + q3 on sync, q1 on scalar, q2 on gpsimd.
            nc.sync.dma_start(out=x_ts[0][:, 0:GC], in_=xgd[:, 0:GC])
            nc.sync.dma_start(out=x_ts[0][:, GC:QW], in_=xgd[:, GC:QW])
            nc.scalar.dma_start(out=x_ts[1], in_=xgd[:, QW:2 * QW])
            nc.gpsimd.dma_start(out=x_ts[2], in_=xgd[:, 2 * QW:3 * QW])
            nc.sync.dma_start(out=x_ts[3], in_=xgd[:, 3 * QW:4 * QW])
            return x_ts

        x_ts = issue_x_loads(first=True)

        # w_x broadcast across partitions: ones[1,128]^T @ wx[1,1]
        wxp = psum.tile([PB, 1], F32, tag="wx")
        nc.tensor.matmul(
            wxp,
            cpk[0:1, CX + B + 2:CX + B + 3].to_broadcast([1, PB]),
            cpk[0:1, CX + B + 1:CX + B + 2],
            start=True,
            stop=True,
        )
        wxc = const.tile([PB, 1], F32)
        nc.vector.tensor_copy(wxc, wxp)

        # ---- ng term on PE: ngp[p, a] = nongene[a*128+p] . W_ng ----
        ngp = psum.tile([PB, RB], F32, tag="ng")
        for a in range(RB):
            nc.tensor.matmul(
                ngp[:, a:a + 1],
                xngT[:, a * PB:(a + 1) * PB],
                wngcol,
                start=True,
                stop=True,
            )
        ngb = const.tile([PB, RB], F32)
        nc.vector.tensor_copy(ngb, ngp)

        # ---- gene term (+ fc bias): matmul + bf16 copy per bank ----
        grow = const.tile([PB, GC], BF16)
        for q in range(NBANK):
            c0 = q * BANK
            cw = min(BANK, GC - c0)
            gps = psum.tile([PB, BANK], F32, tag=f"g{q}")
            nc.tensor.matmul(
                gps[:, 0:cw],
                web,
                embT[:, c0:c0 + cw],
                start=True,
                stop=True,
            )
            nc.vector.tensor_copy(grow[:, c0:c0 + cw], gps[:, 0:cw])

        # ---- main stream: 8 row blocks in 4 quarter tiles ----
        # Block a lives in x_ts[a//2][:, (a%2)*GC :], y pair tiles hold
        # blocks (2k, 2k+1).  ACT does cols [0:SPLIT), Pool the rest,
        # DVE adds grow.  Stores: pair 0,2 -> sync queue (idle engine),
        # pair 1 -> act queue after its compute, pair 3 -> gpsimd queue
        # after its compute.
        lo = slice(0, SPLIT)
        hi = slice(SPLIT, GC)
        for r in range(repeat):
            if r > 0:
                x_ts = issue_x_loads(first=False)
            y_ts = []
            for k in range(NQ):
                y_t = ypool.tile([PB, QW], BF16, tag="y")
                y_ts.append(y_t)
            # phase 1: per-block scale-add on ACT (lo) and Pool (hi)
            for a in range(RB):
                x_blk = x_ts[a // 2][:, (a % 2) * GC:(a % 2 + 1) * GC]
                y_blk = y_ts[a // 2][:, (a % 2) * GC:(a % 2 + 1) * GC]
                nc.scalar.activation(
                    out=y_blk[:, lo],
                    in_=x_blk[:, lo],
                    func=mybir.ActivationFunctionType.Identity,
                    bias=ngb[:, a:a + 1],
                    scale=wxc,
                )
                nc.gpsimd.tensor_scalar(
                    out=y_blk[:, hi],
                    in0=x_blk[:, hi],
                    scalar1=wxc,
                    scalar2=ngb[:, a:a + 1],
                    op0=mybir.AluOpType.mult,
                    op1=mybir.AluOpType.add,
                )
            # phase 2: grow adds on DVE + block-granular stores.
            # Ring balance per iteration stays 2.56 MB each: blocks 0-1 on
            # sync, 2-3 on scalar, 4-7 on gpsimd; finer stores start the
            # write stream ~2us earlier and smooth the HBM demand.
            for a in range(RB):
                y_t = y_ts[a // 2]
                e = a % 2
                for sl in (lo, hi):
                    dst = slice(e * GC + sl.start, e * GC + sl.stop)
                    nc.vector.tensor_add(y_t[:, dst], y_t[:, dst], grow[:, sl])
                if a == 6:
                    continue          # stored together with block 7 below
                if a == 7:
                    # blocks 6+7 as ONE pair store: one less SWDGE
                    # emission on the Q7 (v9 showed emissions cost ~1us)
                    nc.gpsimd.dma_start(
                        out=ysd[:, 6 * GC:8 * GC], in_=y_t
                    )
                    continue
                eng = nc.sync if a < 2 else (nc.scalar if a < 5 else nc.gpsimd)
                eng.dma_start(
                    out=ysd[:, a * GC:(a + 1) * GC],
                    in_=y_t[:, e * GC:(e + 1) * GC],
                )


def make_nc(repeat: int = 1, serial: bool = False) -> bacc.Bacc:
    nc = bacc.Bacc("TRN2", debug=False, num_devices=NCORES)
    build_kernel(nc, repeat=repeat, serial=serial)
    nc.compile()
    return nc


def prep_inputs(inputs) -> list:
    """Shard + downcast + repack the full inputs into per-core in_maps."""
    x = np.asarray(inputs["x"], dtype=np.float32)
    emb = np.asarray(inputs["emb"], dtype=np.float32)
    W = np.asarray(inputs["W"], dtype=np.float32).reshape(FC_IN)
    b = float(np.asarray(inputs["b"], dtype=np.float32).reshape(()))

    base = np.zeros((DNG, CW), dtype=np.float32)
    base[:, CX:CX + B] = x[:, G:].T
    base[:, CX + B] = W[0:DNG]
    base[:, CX + B + 1] = W[DNG]
    base[:, CX + B + 2] = 1.0
    base[0:E, GC:GC + PB] = W[DNG + 1:FC_IN, None]
    base[E, GC:GC + PB] = b

    in_maps = []
    for c in range(NCORES):
        sl = slice(c * GC, (c + 1) * GC)
        cpk = base.copy()
        cpk[0:E, 0:GC] = emb[sl].T
        cpk[E, 0:GC] = 1.0
        xg = (
            np.ascontiguousarray(x[:, sl])
            .reshape(RB, PB, GC)
            .transpose(1, 0, 2)
            .reshape(PB, RB * GC)
        )
        in_maps.append({
            "xg": np.ascontiguousarray(xg).astype(NP_FP8),
            "cpk": cpk.astype(NP_BF16),
        })
    return in_maps


def unshard(res_core: np.ndarray) -> np.ndarray:
    """[128, 8*2500] packed -> [1024, 2500] row-major (pure layout)."""
    return (
        np.asarray(res_core)
        .reshape(PB, RB, GC)
        .transpose(1, 0, 2)
        .reshape(B, GC)
        .astype(np.float32)
    )


def kernel(**inputs) -> np.ndarray:
    nc = make_nc()
    in_maps = prep_inputs(inputs)
    res = run_bass_kernel_spmd(nc, in_maps, core_ids=list(range(NCORES)))
    return np.concatenate(
        [unshard(r["ys"]) for r in res.results], axis=1
    )


# revision 8
# speedup vs baseline: 1.2927x; 1.2291x over previous
"""v13: minimal SWDGE emissions — gpsimd issues ONE 2.56 MB store for
blocks 4-7 per iteration (y tiles restructured to 2 x [128, 10000]);
loads balanced on the two HWDGE rings; every ring still 2.56 MB/iter.
v12 = v8 with blocks 6+7 stored as one pair DMA on gpsimd (3 SWDGE
emissions per iteration instead of 4; same bytes per ring).
v8 = v6 + 3-ring x loads (sync 1.28 / scalar 0.64 / gpsimd 0.64 MB)
and 2/3/3 block stores, so ring totals stay 2.56 MB each while BOTH
the load phase and the store phase use all three DMA queues.
v6 = v4 + block-granular stores (8 x 0.64 MB, same ring balance).
v4 = v2 + balanced DMA rings (2.56 MB each on sync/scalar/gpsimd per
iteration) + block-granular first x load for an earlier compute start.

Math (separable linear):
    y[b, j] = w_x * x[b, j] + (nongene[b] . W_ng + bias) + (emb[j] . W_e)

Sharding: gene-parallel across 8 cores; each core handles a 2500-gene
column slice for the full 1024-row batch.

v2 vs baseline: the one-shot latency is dominated by the serialized DMA
stream (27 small DMAs on one HWDGE ring).  v2 cuts this to 9 DMAs spread
over all three DMA queues (qSPDynamicHW / qActDynamicHW / SWDGE):
  - 1 packed const load on the gpsimd (SWDGE) queue,
  - 4 x quarter-loads [128, 5000] fp8, alternating sync/act queues,
  - 4 y pair-stores [128, 5000] bf16 split sync/act/gpsimd queues,
    issued only at queue positions that never stall a compute engine.
x and y use a col-block = row-block packed DRAM layout ([128, 8*2500])
so row blocks are column ranges of one 128-partition tensor (pure host
layout transform).
"""

import numpy as np
import ml_dtypes
from contextlib import ExitStack

import concourse.bass as bass
import concourse.bacc as bacc
import concourse.tile as tile
from concourse import mybir
from concourse.bass_utils import run_bass_kernel_spmd

F32 = mybir.dt.float32
BF16 = mybir.dt.bfloat16
FP8 = mybir.dt.float8e3

NP_BF16 = ml_dtypes.bfloat16
NP_FP8 = ml_dtypes.float8_e3m4

B = 1024
G = 20000
DNG = 64
E = 32
FC_IN = DNG + 1 + E       # 97
NCORES = 8
GC = G // NCORES          # 2500 gene columns per core
PB = 128                  # SBUF partitions
RB = B // PB              # 8 row blocks per core
NQ = 4                    # x load quarters / y store pairs
QW = RB // NQ * GC        # 5000 cols per quarter/pair
SPLIT = 1536              # ACT handles [0:SPLIT), Pool [SPLIT:GC) per block

BANK = 512                # f32 columns per PSUM bank
NBANK = (GC + BANK - 1) // BANK

# const pack layout: [64, CW] bf16
#   rows 0:33, cols 0:GC        = [embT ; ones]
#   rows 0:33, cols GC:GC+PB    = [W_e | b] broadcast
#   rows 0:64, cols CX:CX+B     = xng.T
#   rows 0:64, col  CX+B        = W_ng
#   rows 0:64, col  CX+B+1      = w_x
#   rows 0:64, col  CX+B+2      = 1.0
CX = GC + PB              # 2628
CW = CX + B + 3           # 3655


def build_kernel(nc: bass.Bass, repeat: int = 1, serial: bool = False):
    xgd = nc.dram_tensor("xg", [PB, RB * GC], FP8, kind="ExternalInput").ap()
    cpkd = nc.dram_tensor("cpk", [DNG, CW], BF16, kind="ExternalInput").ap()
    ysd = nc.dram_tensor("ys", [PB, RB * GC], BF16, kind="ExternalOutput").ap()

    with tile.TileContext(nc) as tc, ExitStack() as ctx:
        const = ctx.enter_context(tc.tile_pool(name="const", bufs=1))
        psum = ctx.enter_context(tc.tile_pool(name="psum", bufs=1, space="PSUM"))
        xpool = ctx.enter_context(tc.tile_pool(name="xpool", bufs=NQ))
        ypool = ctx.enter_context(tc.tile_pool(name="ypool", bufs=2))

        # ---- dummy activation: hoists LoadActFuncSet to t~0 ----
        zin = const.tile([1, 2], F32)
        nc.gpsimd.memset(zin, 0.0)
        zout = const.tile([1, 2], F32)
        nc.scalar.activation(
            out=zout, in_=zin, func=mybir.ActivationFunctionType.Identity
        )

        # ---- const load on the SWDGE queue; x quarters on the two ----
        # ---- HWDGE queues, two each, so all three queues pull at t=0 ----
        cpk = const.tile([DNG, CW], BF16)
        nc.gpsimd.dma_start(out=cpk, in_=cpkd)
        embT = cpk[0:E + 1, 0:GC]               # [33, 2500] = [embT ; ones]
        web = cpk[0:E + 1, GC:GC + PB]          # [33, 128]  = [W_e | b] bcast
        xngT = cpk[:, CX:CX + B]                # [64, 1024] = xng.T
        wngcol = cpk[:, CX + B:CX + B + 1]      # [64, 1]    = W_ng

        gate = const.tile([1, RB], BF16)        # serial-mode barrier tile

        def issue_x_loads(first: bool):
            x_ts = []
            for q in range(NQ):
                x_t = xpool.tile([PB, QW], FP8, tag="x")
                x_ts.append(x_t)
            if serial and not first:
                # gate: strided read touching every stored region, then a
                # tiny DVE splash into each x tile corner.  Forces repeat
                # r's loads to wait for r-1's stores (one-shot latency
                # approximation for the repeat-slope measurement).
                nc.sync.dma_start(
                    out=gate, in_=ysd[0:1, GC - 2:RB * GC:GC]
                )
                for q in range(NQ):
                    nc.vector.tensor_copy(x_ts[q][0:1, 0:RB], gate)
            # loads use ALL THREE rings so the load phase takes 1/3 the
            # ring-time: q0 (split at block granularity for an early
            # compute start) + q3 on sync, q1 on scalar, q2 on gpsimd.
            nc.sync.dma_start(out=x_ts[0][:, 0:GC], in_=xgd[:, 0:GC])
            nc.sync.dma_start(out=x_ts[0][:, GC:QW], in_=xgd[:, GC:QW])
            nc.scalar.dma_start(out=x_ts[1], in_=xgd[:, QW:2 * QW])
            nc.scalar.dma_start(out=x_ts[2], in_=xgd[:, 2 * QW:3 * QW])
            nc.sync.dma_start(out=x_ts[3], in_=xgd[:, 3 * QW:4 * QW])
            return x_ts

        x_ts = issue_x_loads(first=True)

        # w_x broadcast across partitions: ones[1,128]^T @ wx[1,1]
        wxp = psum.tile([PB, 1], F32, tag="wx")
        nc.tensor.matmul(
            wxp,
            cpk[0:1, CX + B + 2:CX + B + 3].to_broadcast([1, PB]),
            cpk[0:1, CX + B + 1:CX + B + 2],
            start=True,
            stop=True,
        )
        wxc = const.tile([PB, 1], F32)
        nc.vector.tensor_copy(wxc, wxp)

        # ---- ng term on PE: ngp[p, a] = nongene[a*128+p] . W_ng ----
        ngp = psum.tile([PB, RB], F32, tag="ng")
        for a in range(RB):
            nc.tensor.matmul(
                ngp[:, a:a + 1],
                xngT[:, a * PB:(a + 1) * PB],
                wngcol,
                start=True,
                stop=True,
            )
        ngb = const.tile([PB, RB], F32)
        nc.vector.tensor_copy(ngb, ngp)

        # ---- gene term (+ fc bias): matmul + bf16 copy per bank ----
        grow = const.tile([PB, GC], BF16)
        for q in range(NBANK):
            c0 = q * BANK
            cw = min(BANK, GC - c0)
            gps = psum.tile([PB, BANK], F32, tag=f"g{q}")
            nc.tensor.matmul(
                gps[:, 0:cw],
                web,
                embT[:, c0:c0 + cw],
                start=True,
                stop=True,
            )
            nc.vector.tensor_copy(grow[:, c0:c0 + cw], gps[:, 0:cw])

        # ---- main stream: 8 row blocks in 4 quarter tiles ----
        # Block a lives in x_ts[a//2][:, (a%2)*GC :], y pair tiles hold
        # blocks (2k, 2k+1).  ACT does cols [0:SPLIT), Pool the rest,
        # DVE adds grow.  Stores: pair 0,2 -> sync queue (idle engine),
        # pair 1 -> act queue after its compute, pair 3 -> gpsimd queue
        # after its compute.
        lo = slice(0, SPLIT)
        hi = slice(SPLIT, GC)
        for r in range(repeat):
            if r > 0:
                x_ts = issue_x_loads(first=False)
            y_ts = []
            for k in range(2):
                y_t = ypool.tile([PB, 2 * QW], BF16, tag="y")
                y_ts.append(y_t)
            # phase 1: per-block scale-add on ACT (lo) and Pool (hi)
            for a in range(RB):
                x_blk = x_ts[a // 2][:, (a % 2) * GC:(a % 2 + 1) * GC]
                y_blk = y_ts[a // 4][:, (a % 4) * GC:(a % 4 + 1) * GC]
                nc.scalar.activation(
                    out=y_blk[:, lo],
                    in_=x_blk[:, lo],
                    func=mybir.ActivationFunctionType.Identity,
                    bias=ngb[:, a:a + 1],
                    scale=wxc,
                )
                nc.gpsimd.tensor_scalar(
                    out=y_blk[:, hi],
                    in0=x_blk[:, hi],
                    scalar1=wxc,
                    scalar2=ngb[:, a:a + 1],
                    op0=mybir.AluOpType.mult,
                    op1=mybir.AluOpType.add,
                )
            # phase 2: grow adds on DVE; stores: blocks 0-1 pair on sync,
            # 2-3 pair on scalar, 4-7 as ONE 2.56 MB SWDGE DMA (a single
            # Q7 emission per iteration — each emission costs ~1 us).
            for a in range(RB):
                y_t = y_ts[a // 4]
                e = a % 4
                for sl in (lo, hi):
                    dst = slice(e * GC + sl.start, e * GC + sl.stop)
                    nc.vector.tensor_add(y_t[:, dst], y_t[:, dst], grow[:, sl])
                if a == 1:
                    nc.sync.dma_start(
                        out=ysd[:, 0:2 * GC], in_=y_t[:, 0:2 * GC]
                    )
                elif a == 3:
                    nc.scalar.dma_start(
                        out=ysd[:, 2 * GC:4 * GC], in_=y_t[:, 2 * GC:4 * GC]
                    )
                elif a == 7:
                    nc.gpsimd.dma_start(out=ysd[:, 4 * GC:8 * GC], in_=y_t)


def make_nc(repeat: int = 1, serial: bool = False) -> bacc.Bacc:
    nc = bacc.Bacc("TRN2", debug=False, num_devices=NCORES)
    build_kernel(nc, repeat=repeat, serial=serial)
    nc.compile()
    return nc


def prep_inputs(inputs) -> list:
    """Shard + downcast + repack the full inputs into per-core in_maps."""
    x = np.asarray(inputs["x"], dtype=np.float32)
    emb = np.asarray(inputs["emb"], dtype=np.float32)
    W = np.asarray(inputs["W"], dtype=np.float32).reshape(FC_IN)
    b = float(np.asarray(inputs["b"], dtype=np.float32).reshape(()))

    base = np.zeros((DNG, CW), dtype=np.float32)
    base[:, CX:CX + B] = x[:, G:].T
    base[:, CX + B] = W[0:DNG]
    base[:, CX + B + 1] = W[DNG]
    base[:, CX + B + 2] = 1.0
    base[0:E, GC:GC + PB] = W[DNG + 1:FC_IN, None]
    base[E, GC:GC + PB] = b

    in_maps = []
    for c in range(NCORES):
        sl = slice(c * GC, (c + 1) * GC)
        cpk = base.copy()
        cpk[0:E, 0:GC] = emb[sl].T
        cpk[E, 0:GC] = 1.0
        xg = (
            np.ascontiguousarray(x[:, sl])
            .reshape(RB, PB, GC)
            .transpose(1, 0, 2)
            .reshape(PB, RB * GC)
        )
        in_maps.append({
            "xg": np.ascontiguousarray(xg).astype(NP_FP8),
            "cpk": cpk.astype(NP_BF16),
        })
    return in_maps


def unshard(res_core: np.ndarray) -> np.ndarray:
    """[128, 8*2500] packed -> [1024, 2500] row-major (pure layout)."""
    return (
        np.asarray(res_core)
        .reshape(PB, RB, GC)
        .transpose(1, 0, 2)
        .reshape(B, GC)
        .astype(np.float32)
    )


def kernel(**inputs) -> np.ndarray:
    nc = make_nc()
    in_maps = prep_inputs(inputs)
    res = run_bass_kernel_spmd(nc, in_maps, core_ids=list(range(NCORES)))
    return np.concatenate(
        [unshard(r["ys"]) for r in res.results], axis=1
    )
